# revision 3
# baseline (speedup 1.0000x reference)
"""MultiHeadLatentAttention (MLA) Trainium2 kernel — 8-core SPMD, tensor-parallel over heads.

Strategy (per core c, owning heads 2c and 2c+1):
  - Q path fused on host: Wq_h = wq_up_h @ diag(q_norm_w) @ wq_down (the rmsnorm scale
    alpha_t commutes through the linear up-projection).  alpha_t itself needs
    ||x @ wq_down.T||^2 over all 1536 ranks: each core computes a 192-rank shard of the
    sum of squares and a tiny [1, 2048] AllReduce (one per batch) completes it; both
    collectives are hidden under compute.
  - KV path: wkv_down replicated (small), wkv_up sharded by head (kv_norm folded in),
    beta_t (kv rmsnorm scale) computed locally and applied at PSUM eviction.
  - All tensor-engine matmuls in bf16 (FWL enabled; f32 PSUM accumulation); the
    norm scales alpha/beta and softmax denominators stay f32.
  - Attention computed in transposed layout S^T[k, q] so P@V needs no transposes;
    softmax denominator via DVE accumulation + gpsimd partition_all_reduce; no max
    subtraction (scores are O(5), exp is safe in fp32).
  - Front-phase results staged through local DRAM to keep SBUF under budget.
  - wo applied per core to its 2 heads; host sums the 8 partial [2048, 4096] outputs.
"""

import math
import numpy as np

import concourse.bacc as bacc
import concourse.mybir as mybir
import concourse.tile as tile
from concourse.bass_utils import run_bass_kernel_spmd

F32 = mybir.dt.float32
BF16 = mybir.dt.bfloat16
NPBF16 = mybir.dt.np(mybir.dt.bfloat16)

N_CORES = 8
HPC = 2               # heads per core
DIM = 2048
NH = 16
QR = 1536
KVR = 512
DN = 128
DR = 64
DV = 128
B = 2
S = 2048
T = B * S
EPS = 1e-6
SCALE = 1.0 / math.sqrt(DN + DR)
ROPE_THETA = 10000.0

TB = 256              # front token block
NTB = S // TB         # 8 blocks per batch
DCH = DIM // 128      # 16 contraction chunks
RSH = QR // N_CORES   # 192-rank ss shard per core

_BUILD_CACHE = {}


def _build_program(reps=1):
    if ("nc", reps) in _BUILD_CACHE:
        return _BUILD_CACHE[("nc", reps)]

    nc = bacc.Bacc(num_devices=N_CORES)

    # ---------------- DRAM I/O ----------------
    xT_d = nc.dram_tensor("xT", [B, DIM, S], BF16, kind="ExternalInput")
    wqss_d = nc.dram_tensor("wqss", [DIM, RSH], BF16, kind="ExternalInput")
    wq_d = nc.dram_tensor("wq", [DIM, HPC * DN], BF16, kind="ExternalInput")
    wqp_d = nc.dram_tensor("wqp", [DIM, HPC * DR], BF16, kind="ExternalInput")
    wkvd_d = nc.dram_tensor("wkvd", [DIM, KVR], BF16, kind="ExternalInput")
    wkvu_d = nc.dram_tensor("wkvu", [KVR, HPC * (DN + DV)], BF16, kind="ExternalInput")
    wkr_d = nc.dram_tensor("wkr", [DIM, DR], BF16, kind="ExternalInput")
    wo0_d = nc.dram_tensor("wo0", [DV, DIM], BF16, kind="ExternalInput")
    wo1_d = nc.dram_tensor("wo1", [DV, DIM], BF16, kind="ExternalInput")
    ctab_d = nc.dram_tensor("ctab", [128, S], F32, kind="ExternalInput")
    stab_d = nc.dram_tensor("stab", [128, S], F32, kind="ExternalInput")
    masks_d = nc.dram_tensor("masks", [128, 4 * 512], BF16, kind="ExternalInput")
    ident_d = nc.dram_tensor("ident", [128, 128], BF16, kind="ExternalInput")

    yT_d = nc.dram_tensor("yT", [DIM, T], F32, kind="ExternalOutput")

    # ---------------- internal DRAM scratch ----------------
    qn_s = [[nc.dram_tensor(f"qn_{b}_{h}", [DN, S], BF16) for h in range(HPC)] for b in range(B)]
    qp_s = [nc.dram_tensor(f"qp_{b}", [HPC * DR, S], BF16) for b in range(B)]
    kn_s = [[nc.dram_tensor(f"kn_{b}_{h}", [DN, S], BF16) for h in range(HPC)] for b in range(B)]
    v_s = [[nc.dram_tensor(f"v_{b}_{h}", [S, DV], BF16) for h in range(HPC)] for b in range(B)]
    kr_s = [nc.dram_tensor(f"kr_{b}", [DR, S], BF16) for b in range(B)]
    ssin = [nc.dram_tensor(f"ssin_{b}", [1, S], F32) for b in range(B)]
    ssout = [nc.dram_tensor(f"ssout_{b}", [1, S], F32, addr_space="Shared") for b in range(B)]

    import concourse.bass_isa as bass_isa
    RADD = bass_isa.ReduceOp.add

    with tile.TileContext(nc) as tc:
        with tc.tile_pool(name="wpool", bufs=1) as wp:
            # resident weights / constants
            wq_t = wp.tile([128, DCH, HPC * DN], BF16, tag="wq")
            wqp_t = wp.tile([128, DCH, HPC * DR], BF16, tag="wqp")
            wkvu_t = wp.tile([128, KVR // 128, HPC * (DN + DV)], BF16, tag="wkvu")
            wkr_t = wp.tile([128, DCH, DR], BF16, tag="wkr")
            wo_t = [wp.tile([DV, DIM], BF16, tag=f"wo{h}", name=f"wo_t{h}") for h in range(HPC)]
            masks_t = wp.tile([128, 4 * 512], BF16, tag="masks")
            ident_t = wp.tile([128, 128], BF16, tag="ident")
            nc.sync.dma_start(ident_t[:], ident_d[:])
            eps_t = wp.tile([128, 1], F32, tag="eps")
            nc.gpsimd.memset(eps_t[:], EPS)

            # ======================= FRONT PHASE =======================
            for rep in range(reps):
                with tc.tile_pool(name="fpool", bufs=1) as fp, \
                   tc.tile_pool(name="fps", bufs=1, space="PSUM") as fpp:
                  wqss_t = fp.tile([128, DCH, RSH], BF16, tag="wqss")
                  nc.sync.dma_start(wqss_t[:], wqss_d.ap().rearrange("(c p) m -> p c m", p=128))
                  wkvd_t = fp.tile([128, DCH, KVR], BF16, tag="wkvd")
                  nc.sync.dma_start(wkvd_t[:], wkvd_d.ap().rearrange("(c p) m -> p c m", p=128))
                  ctab_t = fp.tile([128, S], F32, tag="ctab")
                  nc.sync.dma_start(ctab_t[:], ctab_d[:])
                  stab_t = fp.tile([128, S], F32, tag="stab")
                  nc.sync.dma_start(stab_t[:], stab_d[:])
                  if rep == 0:
                      nc.sync.dma_start(wkvu_t[:], wkvu_d.ap().rearrange("(c p) m -> p c m", p=128))
                      nc.sync.dma_start(wkr_t[:], wkr_d.ap().rearrange("(c p) m -> p c m", p=128))
                      nc.sync.dma_start(wq_t[:], wq_d.ap().rearrange("(c p) m -> p c m", p=128))
                      nc.sync.dma_start(wqp_t[:], wqp_d.ap().rearrange("(c p) m -> p c m", p=128))
                  for b in range(B):
                      ss_row = fp.tile([1, S], F32, tag="ss_row")
                      for j in range(NTB):
                          t0 = j * TB
                          xt = fp.tile([128, DCH, TB], BF16, tag="xt", bufs=2)
                          nc.sync.dma_start(
                              xt[:], xT_d.ap()[b, :, t0:t0 + TB].rearrange("(c p) t -> p c t", p=128))

                          # ---- ss shard (raw q_c norm partial) ----
                          ps_a = fpp.tile([128, TB], F32, tag="p128", bufs=4)
                          for d in range(DCH):
                              nc.tensor.matmul(ps_a[:], wqss_t[:, d, 0:128], xt[:, d, :],
                                               start=(d == 0), stop=(d == DCH - 1))
                          ps_b = fpp.tile([64, TB], F32, tag="p64", bufs=2)
                          for d in range(DCH):
                              nc.tensor.matmul(ps_b[:], wqss_t[:, d, 128:192], xt[:, d, :],
                                               start=(d == 0), stop=(d == DCH - 1))
                          sq_a = fp.tile([128, TB], F32, tag="sq_a", bufs=2)
                          nc.scalar.activation(sq_a[:], ps_a[:], mybir.ActivationFunctionType.Square)
                          sq_b = fp.tile([64, TB], F32, tag="sq_b", bufs=2)
                          nc.scalar.activation(sq_b[:], ps_b[:], mybir.ActivationFunctionType.Square)
                          pr_a = fp.tile([128, TB], F32, tag="pr_a", bufs=1)
                          nc.gpsimd.partition_all_reduce(pr_a[:], sq_a[:], channels=128, reduce_op=RADD)
                          pr_b = fp.tile([64, TB], F32, tag="pr_b", bufs=1)
                          nc.gpsimd.partition_all_reduce(pr_b[:], sq_b[:], channels=64, reduce_op=RADD)
                          nc.vector.tensor_add(ss_row[0:1, t0:t0 + TB], pr_a[0:1, :], pr_b[0:1, :])

                          # ---- kv_c (4 rank chunks) + beta ----
                          kvc = fp.tile([128, KVR // 128, TB], BF16, tag="kvc", bufs=2)
                          sqk = fp.tile([128, TB], F32, tag="sqk", bufs=2)
                          for rc in range(KVR // 128):
                              ps_kv = fpp.tile([128, TB], F32, tag="p128", bufs=4)
                              for d in range(DCH):
                                  nc.tensor.matmul(ps_kv[:], wkvd_t[:, d, rc * 128:(rc + 1) * 128],
                                                   xt[:, d, :], start=(d == 0), stop=(d == DCH - 1))
                              nc.vector.tensor_copy(kvc[:, rc, :], ps_kv[:])
                              if rc == 0:
                                  nc.scalar.activation(sqk[:], ps_kv[:], mybir.ActivationFunctionType.Square)
                              else:
                                  sqk2 = fp.tile([128, TB], F32, tag="sqk2", bufs=2)
                                  nc.scalar.activation(sqk2[:], ps_kv[:], mybir.ActivationFunctionType.Square)
                                  nc.vector.tensor_add(sqk[:], sqk[:], sqk2[:])
                          prk = fp.tile([128, TB], F32, tag="prk", bufs=2)
                          nc.gpsimd.partition_all_reduce(prk[:], sqk[:], channels=128, reduce_op=RADD)
                          # beta = 1/sqrt(mean + eps), on row 0 then broadcast
                          brow = fp.tile([1, TB], F32, tag="brow", bufs=2)
                          nc.scalar.activation(brow[:], prk[0:1, :], mybir.ActivationFunctionType.Sqrt,
                                               scale=1.0 / KVR, bias=eps_t[0:1, :])
                          nc.vector.reciprocal(brow[:], brow[:])
                          bbc = fp.tile([128, TB], F32, tag="bbc", bufs=2)
                          nc.gpsimd.partition_broadcast(bbc[:], brow[:])

                          # ---- kv up-projection (K0 V0 K1 V1), scaled by beta ----
                          for m in range(4):  # 0: K h0, 1: V h0, 2: K h1, 3: V h1
                              h, is_v = m // 2, m % 2
                              ps_up = fpp.tile([128, TB], F32, tag="p128", bufs=4)
                              for rc in range(KVR // 128):
                                  nc.tensor.matmul(ps_up[:], wkvu_t[:, rc, m * 128:(m + 1) * 128],
                                                   kvc[:, rc, :], start=(rc == 0), stop=(rc == 3))
                              stg = fp.tile([128, TB], BF16, tag="stg_up", bufs=2)
                              nc.vector.tensor_mul(stg[:], ps_up[:], bbc[:])
                              if not is_v:
                                  nc.sync.dma_start(kn_s[b][h].ap()[:, t0:t0 + TB], stg[:])
                              else:
                                  # transpose to natural [t, dv] layout
                                  for c2 in range(TB // 128):
                                      tps = fpp.tile([128, 128], BF16, tag="ptp", bufs=2)
                                      nc.tensor.transpose(tps[:], stg[:, c2 * 128:(c2 + 1) * 128], ident_t[:])
                                      vn = fp.tile([128, 128], BF16, tag="vn", bufs=2)
                                      nc.vector.tensor_copy(vn[:], tps[:])
                                      nc.sync.dma_start(
                                          v_s[b][h].ap()[t0 + c2 * 128: t0 + (c2 + 1) * 128, :], vn[:])

                          # ---- k_rope (shared head) + rope rotation ----
                          ps_kr = fpp.tile([64, TB], F32, tag="p64", bufs=2)
                          for d in range(DCH):
                              nc.tensor.matmul(ps_kr[:], wkr_t[:, d, :], xt[:, d, :],
                                               start=(d == 0), stop=(d == DCH - 1))
                          tmp = fp.tile([64, TB], F32, tag="krtmp", bufs=2)
                          nc.vector.tensor_copy(tmp[0:32, :], ps_kr[32:64, :])
                          nc.vector.tensor_copy(tmp[32:64, :], ps_kr[0:32, :])
                          krr = fp.tile([64, TB], BF16, tag="krr", bufs=2)
                          m1 = fp.tile([64, TB], F32, tag="krm1", bufs=2)
                          nc.vector.tensor_mul(m1[:], ps_kr[:], ctab_t[0:64, t0:t0 + TB])
                          nc.vector.tensor_mul(tmp[:], tmp[:], stab_t[0:64, t0:t0 + TB])
                          nc.vector.tensor_add(krr[:], m1[:], tmp[:])
                          nc.sync.dma_start(kr_s[b].ap()[:, t0:t0 + TB], krr[:])

                          # ---- Qn raw (2 heads) ----
                          for h in range(HPC):
                              ps_qn = fpp.tile([128, TB], F32, tag="p128", bufs=4)
                              for d in range(DCH):
                                  nc.tensor.matmul(ps_qn[:], wq_t[:, d, h * DN:(h + 1) * DN],
                                                   xt[:, d, :], start=(d == 0), stop=(d == DCH - 1))
                              qstg = fp.tile([128, TB], BF16, tag="qstg", bufs=2)
                              nc.vector.tensor_copy(qstg[:], ps_qn[:])
                              nc.sync.dma_start(qn_s[b][h].ap()[:, t0:t0 + TB], qstg[:])

                          # ---- Qp raw (2 heads stacked) + rope ----
                          ps_qp = fpp.tile([128, TB], F32, tag="p128", bufs=4)
                          for d in range(DCH):
                              nc.tensor.matmul(ps_qp[:], wqp_t[:, d, :], xt[:, d, :],
                                               start=(d == 0), stop=(d == DCH - 1))
                          qtmp = fp.tile([128, TB], F32, tag="qptmp", bufs=2)
                          for h in range(HPC):
                              o = h * 64
                              nc.vector.tensor_copy(qtmp[o:o + 32, :], ps_qp[o + 32:o + 64, :])
                              nc.vector.tensor_copy(qtmp[o + 32:o + 64, :], ps_qp[o:o + 32, :])
                          qm1 = fp.tile([128, TB], F32, tag="qpm1", bufs=2)
                          nc.vector.tensor_mul(qm1[:], ps_qp[:], ctab_t[:, t0:t0 + TB])
                          nc.vector.tensor_mul(qtmp[:], qtmp[:], stab_t[:, t0:t0 + TB])
                          qrot = fp.tile([128, TB], BF16, tag="qrot", bufs=2)
                          nc.vector.tensor_add(qrot[:], qm1[:], qtmp[:])
                          nc.sync.dma_start(qp_s[b].ap()[:, t0:t0 + TB], qrot[:])

                      # stage this batch's ss; b0's AllReduce kicks now (hidden
                      # under front b1), b1's kicks during attention b0 so the
                      # Pool-blocking CC never gates the alpha broadcasts.
                      nc.sync.dma_start(ssin[b][:], ss_row[:])
                      nc.gpsimd.collective_compute(
                          "AllReduce", mybir.AluOpType.add,
                          replica_groups=[list(range(N_CORES))],
                          ins=[ssin[b][:]], outs=[ssout[b][:]],
                      )

                # ======================= ATTENTION PHASE =======================
                if rep == 0:
                    nc.sync.dma_start(wo_t[0][:], wo0_d[:])
                    nc.sync.dma_start(wo_t[1][:], wo1_d[:])
                    nc.sync.dma_start(masks_t[:], masks_d[:])
                with tc.tile_pool(name="apool", bufs=1) as ap, \
                   tc.tile_pool(name="aps", bufs=1, space="PSUM") as app:
                  for b in range(B):
                      # alpha = 1/sqrt(ss/QR + eps), broadcast to 128 partitions
                      ssr = ap.tile([1, S], F32, tag="ssr")
                      nc.sync.dma_start(ssr[:], ssout[b][:])
                      arow = ap.tile([1, S], F32, tag="arow")
                      nc.scalar.activation(arow[:], ssr[:], mybir.ActivationFunctionType.Sqrt,
                                           scale=1.0 / QR, bias=eps_t[0:1, :])
                      nc.vector.reciprocal(arow[:], arow[:])
                      abc = ap.tile([128, S], F32, tag="abc")
                      nc.gpsimd.partition_broadcast(abc[:], arow[:])

                      kr_sb = ap.tile([64, S], BF16, tag="kr_sb")
                      for ck in range(4):
                          nc.sync.dma_start(kr_sb[:, ck * 512:(ck + 1) * 512],
                                            kr_s[b].ap()[:, ck * 512:(ck + 1) * 512])

                      out_sb = [ap.tile([128, S], BF16, tag=f"out{h}", name=f"out_sb{h}") for h in range(HPC)]

                      for h in range(HPC):
                          kn_sb = ap.tile([128, S], BF16, tag="kn_sb", bufs=2)
                          v_sb = ap.tile([128, S // 128, DV], BF16, tag="v_sb", bufs=2)
                          for ck in range(4):
                              nc.sync.dma_start(kn_sb[:, ck * 512:(ck + 1) * 512],
                                                kn_s[b][h].ap()[:, ck * 512:(ck + 1) * 512])
                              nc.sync.dma_start(
                                  v_sb[:, ck * 4:(ck + 1) * 4, :],
                                  v_s[b][h].ap()[ck * 512:(ck + 1) * 512, :].rearrange("(c p) v -> p c v", p=128))

                          for qt in range(4):
                              q0 = qt * 512
                              nkc = 4 * (qt + 1)
                              qn_t = ap.tile([128, 512], BF16, tag="qn_t", bufs=2)
                              nc.sync.dma_start(qn_t[:], qn_s[b][h].ap()[:, q0:q0 + 512])
                              qn_sc = ap.tile([128, 512], BF16, tag="qn_sc", bufs=2)
                              nc.vector.tensor_mul(qn_sc[:], qn_t[:], abc[:, q0:q0 + 512])
                              qp_t = ap.tile([64, 512], BF16, tag="qp_t", bufs=2)
                              nc.sync.dma_start(qp_t[:], qp_s[b].ap()[h * 64:(h + 1) * 64, q0:q0 + 512])
                              qp_sc = ap.tile([64, 512], BF16, tag="qp_sc", bufs=2)
                              nc.vector.tensor_mul(qp_sc[:], qp_t[:], abc[0:64, q0:q0 + 512])

                              O = app.tile([128, 512], F32, tag="pO", bufs=2)
                              l_acc = ap.tile([128, 512], F32, tag="l_acc", bufs=2)
                              for kc in range(nkc):
                                  k0 = kc * 128
                                  s_ps = app.tile([128, 512], F32, tag="ps_s", bufs=3)
                                  nc.tensor.matmul(s_ps[:], kn_sb[:, k0:k0 + 128], qn_sc[:],
                                                   start=True, stop=False)
                                  nc.tensor.matmul(s_ps[:], kr_sb[:, k0:k0 + 128], qp_sc[:],
                                                   start=False, stop=True)
                                  P = ap.tile([128, 512], BF16, tag="P", bufs=4)
                                  nc.scalar.activation(P[:], s_ps[:], mybir.ActivationFunctionType.Exp,
                                                       scale=SCALE)
                                  if kc >= 4 * qt:
                                      mi = kc - 4 * qt
                                      nc.vector.tensor_mul(P[:], P[:],
                                                           masks_t[:, mi * 512:(mi + 1) * 512])
                                  if kc == 0:
                                      nc.vector.tensor_copy(l_acc[:], P[:])
                                  else:
                                      nc.vector.tensor_add(l_acc[:], l_acc[:], P[:])
                                  nc.tensor.matmul(O[:], v_sb[:, kc, :], P[:],
                                                   start=(kc == 0), stop=(kc == nkc - 1))
                              l_bc = ap.tile([128, 512], F32, tag="l_bc", bufs=2)
                              nc.gpsimd.partition_all_reduce(l_bc[:], l_acc[:], channels=128, reduce_op=RADD)
                              nc.vector.reciprocal(l_bc[:], l_bc[:])
                              nc.vector.tensor_mul(out_sb[h][:, q0:q0 + 512], O[:], l_bc[:])

                      # wo for this batch
                      for qt in range(4):
                          q0 = qt * 512
                          for dm in range(DCH):
                              y_ps = app.tile([128, 512], F32, tag="py", bufs=2)
                              nc.tensor.matmul(y_ps[:], wo_t[0][:, dm * 128:(dm + 1) * 128],
                                               out_sb[0][:, q0:q0 + 512], start=True, stop=False)
                              nc.tensor.matmul(y_ps[:], wo_t[1][:, dm * 128:(dm + 1) * 128],
                                               out_sb[1][:, q0:q0 + 512], start=False, stop=True)
                              y_sb = ap.tile([128, 512], F32, tag="y_sb", bufs=3)
                              nc.vector.tensor_copy(y_sb[:], y_ps[:])
                              nc.sync.dma_start(
                                  yT_d.ap()[dm * 128:(dm + 1) * 128, b * S + q0: b * S + q0 + 512],
                                  y_sb[:])

    nc.finalize()
    _BUILD_CACHE[("nc", reps)] = nc
    return nc


def _host_inputs(x, wq_down, q_norm_w, wq_up, wq_rope, wkv_down, kv_norm_w, wkv_up, wk_rope, wo):
    """Build the 8 per-core input maps."""
    f32 = np.float32
    x = np.asarray(x, f32)
    xT = np.ascontiguousarray(np.transpose(x, (0, 2, 1))).astype(NPBF16)   # [B, DIM, S]

    p64 = np.concatenate([np.arange(0, DR, 2), np.arange(1, DR, 2)])  # deinterleave

    wq_down_n = (np.asarray(q_norm_w, f32)[:, None] * np.asarray(wq_down, f32))  # [QR, DIM]
    wkv_up_eff = np.asarray(wkv_up, f32) * np.asarray(kv_norm_w, f32)[None, :]   # [NH*(DN+DV), KVR]

    # rope tables (deinterleaved convention), stacked x2 for the two heads
    inv_freq = (1.0 / (ROPE_THETA ** (np.arange(0, DR, 2, dtype=np.float64) / DR)))  # [32]
    ang = np.arange(S, dtype=np.float64)[:, None] * inv_freq[None, :]                # [S, 32]
    cos_t, sin_t = np.cos(ang), np.sin(ang)
    C64 = np.concatenate([cos_t.T, cos_t.T], axis=0).astype(f32)                     # [64, S]
    S64 = np.concatenate([-sin_t.T, sin_t.T], axis=0).astype(f32)                    # [64, S]
    ctab = np.concatenate([C64, C64], axis=0)                                        # [128, S]
    stab = np.concatenate([S64, S64], axis=0)

    # causal masks for the 4 diagonal offsets
    kr = np.arange(128)[:, None]
    qr = np.arange(512)[None, :]
    masks = np.concatenate(
        [(kr + off <= qr).astype(f32) for off in (0, 128, 256, 384)], axis=1).astype(NPBF16)

    ident = np.eye(128, dtype=f32).astype(NPBF16)

    in_maps = []
    for c in range(N_CORES):
        h0, h1 = HPC * c, HPC * c + 1
        wq_blocks, wqp_blocks, wkvu_cols, wo_list = [], [], [], []
        for h in (h0, h1):
            wq_blocks.append(np.asarray(wq_up, f32)[h * DN:(h + 1) * DN, :] @ wq_down_n)
            wr = np.asarray(wq_rope, f32)[h * DR:(h + 1) * DR, :][p64, :]
            wqp_blocks.append(wr @ wq_down_n)
            wkvu_cols.append(wkv_up_eff[h * (DN + DV): h * (DN + DV) + DN, :].T)      # K_h  [KVR, DN]
            wkvu_cols.append(wkv_up_eff[h * (DN + DV) + DN: (h + 1) * (DN + DV), :].T)  # V_h
            wo_list.append(np.ascontiguousarray(np.asarray(wo, f32)[:, h * DV:(h + 1) * DV].T))
        in_maps.append({
            "xT": xT,
            "wqss": np.ascontiguousarray(np.asarray(wq_down, f32)[c * RSH:(c + 1) * RSH, :].T).astype(NPBF16),
            "wq": np.ascontiguousarray(np.concatenate(wq_blocks, axis=0).T).astype(NPBF16),
            "wqp": np.ascontiguousarray(np.concatenate(wqp_blocks, axis=0).T).astype(NPBF16),
            "wkvd": np.ascontiguousarray(np.asarray(wkv_down, f32).T).astype(NPBF16),
            "wkvu": np.ascontiguousarray(np.concatenate(wkvu_cols, axis=1)).astype(NPBF16),
            "wkr": np.ascontiguousarray(np.asarray(wk_rope, f32)[p64, :].T).astype(NPBF16),
            "wo0": wo_list[0].astype(NPBF16),
            "wo1": wo_list[1].astype(NPBF16),
            "ctab": ctab,
            "stab": stab,
            "masks": masks,
            "ident": ident,
        })
    return in_maps


def kernel(**inputs) -> np.ndarray:
    nc = _build_program(1)
    in_maps = _host_inputs(**inputs)
    res = run_bass_kernel_spmd(nc, in_maps, core_ids=list(range(N_CORES)))
    yT = np.zeros((DIM, T), np.float32)
    for c in range(N_CORES):
        yT += res.results[c]["yT"]
    return np.ascontiguousarray(yT.T.reshape(B, S, DIM))
